# revision 1
# baseline (speedup 1.0000x reference)
"""Chi2 loss over ragged windows — Trainium2 Bass kernel.

Math (per sample b of B=4096, rows of length L=4096):
    len  = e_in - s_in            (in [1024, 3072])
    chi2 = sum_{j<len} ivar[b, s_in+j] * (flu[b, s_in+j] - out[b, s_out+j])^2
    result = mean_b(chi2 / len)

Strategy: pure data-parallel over the batch, 512 samples per core on 8
cores. The three arrays are concatenated into one DRAM tensor per core;
each 128-sample tile is fetched with a single indirect DMA (3 window
chunks per sample row, offsets precomputed on host), aligned so position
j holds flu[s_in+j] / ivar[s_in+j] / out[s_out+j]. On-chip: d = x - y,
d2 = d^2 (ACT), a j < len mask zeroes the ragged tail, prod = d2 * w *
mask, and a per-partition reduction produces one partial sum per sample.
Host divides by len and takes the global mean.

Perf shape (from cost-model timeline iteration):
  - samples sorted by len inside each core; tile t's gather is only as
    wide as its longest window (rounded to 128, shared across cores for
    the single SPMD program) — ~25% less HBM traffic.
  - each tile is split into a maskless "base" piece (columns below the
    tile's min len, always valid) and a masked "tail" piece.
  - masked tail pieces transfer first (high compute/byte), maskless
    bases last, so the DVE never accumulates a backlog and the exposed
    tail after the final transfer is one short chain.
  - the final base piece is split in two and the last two pieces compute
    entirely on the DVE (square/reduce instead of ACT) to avoid
    cross-engine semaphore hops in the drain.
  - SWDGE descriptor ring enlarged (32KB/partition) so descriptor
    generation runs arbitrarily far ahead of the transfers.
"""

import numpy as np

import bass_rust
import concourse.bass as bass
import concourse.tile as tile
from concourse import mybir
from concourse.bass_utils import run_bass_kernel_spmd
from concourse.tile_rust import add_dep_helper

B, L = 4096, 4096
N_CORES = 8
BPC = B // N_CORES          # samples per core
P = 128                     # SBUF partitions
TILES = BPC // P            # 128-sample tiles per core
MAX_W = 3072                # max window length
ROWS = 3 * (BPC + 1)        # concat of flu/ivr/oup shards, each padded 1 row

f32 = mybir.dt.float32
i32 = mybir.dt.int32


def legalize_waits(nc):
    """This compiler build only accepts one sync wait per instruction; hoist
    extra waits into standalone single-wait EventSemaphore instructions."""
    n = 0
    for func in nc.m.functions:
        for blk in func.blocks:
            insts = blk.instructions
            out = []
            for inst in insts:
                si = inst.sync_info
                if si is not None and si.on_wait and len(si.on_wait) > 1:
                    waits = list(si.on_wait)
                    for w in waits[:-1]:
                        n += 1
                        out.append(
                            bass_rust.InstEventSemaphore(
                                name=f"splitwait_{n}_{inst.name}",
                                engine=inst.engine,
                                ins=[],
                                outs=[],
                                sync_info=mybir.SyncInfo(on_wait=[w], on_update=[]),
                            )
                        )
                    inst.sync_info = mybir.SyncInfo(
                        on_wait=[waits[-1]], on_update=list(si.on_update)
                    )
                out.append(inst)
            if len(out) != len(insts):
                blk.instructions[:] = out
    return n


def make_work(widths, bases, split_last_base=2):
    """Work items (t, lo, hi, masked, col): masked tails first, bases last,
    the final base split for a short exposed drain."""
    tails = []
    base_pieces = []
    col = 0
    for t in range(TILES):
        W = widths[t]
        bs = bases[t]
        if W > bs:
            tails.append((t, bs, W, True, col))
            col += 1
    last_t = None
    for t in range(TILES):
        if bases[t] > 0:
            last_t = t
    for t in range(TILES):
        bs = bases[t]
        if bs <= 0:
            continue
        if t == last_t and split_last_base > 1 and bs >= 256:
            h = (bs // split_last_base) // 128 * 128
            h = max(h, 128)
            cuts = list(range(0, bs, h))
            for i, lo in enumerate(cuts):
                hi = bs if i == len(cuts) - 1 else min(bs, lo + h)
                if hi > lo:
                    base_pieces.append((t, lo, hi, False, col))
                    col += 1
        else:
            base_pieces.append((t, 0, bs, False, col))
            col += 1
    # interleave masked tails with maskless bases: spreads the compute-heavy
    # pieces across the transfer stream (measured best in the cost model)
    out = []
    for i in range(max(len(tails), len(base_pieces))):
        if i < len(tails):
            out.append(tails[i])
        if i < len(base_pieces):
            out.append(base_pieces[i])
    return out, col


def build_bass(widths, bases, dve_only_last=1, io_bufs=None, m_bufs=None,
               scratch=32768):
    work, ncol = make_work(widths, bases)

    # size pools to fit SBUF for any piece structure
    wp = max((hi - lo) for (_, lo, hi, _, _) in work)
    wm = max(((hi - lo) for (_, lo, hi, mk, _) in work if mk), default=1)
    budget = 148 * 1024 - (MAX_W * 4)
    if m_bufs is None:
        m_bufs = 4 if wm * 4 * 4 <= 40 * 1024 else 2
    if io_bufs is None:
        io_bufs = max(2, min(4, (budget - m_bufs * wm * 4) // (3 * wp * 4)))

    nc = bass.Bass(dynamic_dma_scratch_size=scratch)

    dat = nc.dram_tensor("dat", [ROWS, L], f32, kind="ExternalInput")
    idx = nc.dram_tensor("idx", [P, 3 * TILES], i32, kind="ExternalInput")
    lens = nc.dram_tensor("lens", [P, TILES], f32, kind="ExternalInput")
    res = nc.dram_tensor("res", [P, max(ncol, 1)], f32, kind="ExternalOutput")

    iota_base = min([lo for (_, lo, hi, m, _) in work if m], default=0)

    with tile.TileContext(nc) as tc:
        with (
            tc.tile_pool(name="sc", bufs=1) as sc,
            tc.tile_pool(name="io", bufs=io_bufs) as io,
            tc.tile_pool(name="mp", bufs=m_bufs) as mp,
        ):
            idx_sb = sc.tile([P, 3 * TILES], i32)
            len_sb = sc.tile([P, TILES], f32)
            acc = sc.tile([P, max(ncol, 1)], f32)
            iw = max(MAX_W - iota_base, 1)
            iota_f = sc.tile([P, iw], f32)

            idx_dma = nc.sync.dma_start(out=idx_sb[:], in_=idx[:])
            nc.sync.dma_start(out=len_sb[:], in_=lens[:])

            def emit_gather(t, lo, hi):
                # one single-index gather per array: HW SWDGE reads exactly one
                # offset per partition (multi-index offset tables read as the
                # sim suggests do NOT work on hardware)
                tiles3 = []
                for a, tag in ((0, "x"), (1, "w"), (2, "y")):
                    ti = io.tile([P, hi - lo], f32, tag=tag)
                    nc.gpsimd.indirect_dma_start(
                        out=ti[:], out_offset=None, in_=dat[:],
                        in_offset=bass.IndirectOffsetOnAxis(
                            ap=idx_sb[:, 3 * t + a : 3 * t + a + 1], axis=1
                        ),
                        element_offset=lo,
                    )
                    tiles3.append(ti)
                return tiles3

            def emit_compute(t, g, lo, hi, masked, acc_col, dve_only):
                x = g[0][:]
                w_ = g[1][:]
                y = g[2][:]
                nc.vector.tensor_tensor(
                    out=x, in0=x, in1=y, op=mybir.AluOpType.subtract
                )
                if dve_only:
                    nc.vector.tensor_tensor(
                        out=y, in0=x, in1=x, op=mybir.AluOpType.mult
                    )
                else:
                    nc.scalar.activation(
                        out=y, in_=x, func=mybir.ActivationFunctionType.Square
                    )
                if masked:
                    m = mp.tile([P, hi - lo], f32, tag="m")
                    nc.vector.tensor_scalar(
                        out=m[:],
                        in0=iota_f[:, lo - iota_base : hi - iota_base],
                        scalar1=len_sb[:, t : t + 1],
                        scalar2=None,
                        op0=mybir.AluOpType.is_lt,
                    )
                    nc.vector.tensor_tensor(
                        out=m[:], in0=w_[:], in1=m[:], op=mybir.AluOpType.mult
                    )
                    nc.vector.tensor_tensor(
                        out=w_[:], in0=y[:], in1=m[:], op=mybir.AluOpType.mult
                    )
                else:
                    nc.vector.tensor_tensor(
                        out=w_[:], in0=y[:], in1=w_[:], op=mybir.AluOpType.mult
                    )
                if dve_only:
                    nc.vector.tensor_reduce(
                        out=acc[:, acc_col : acc_col + 1], in_=w_[:],
                        axis=mybir.AxisListType.X, op=mybir.AluOpType.add,
                    )
                else:
                    nc.scalar.activation(
                        out=x, in_=w_[:],
                        func=mybir.ActivationFunctionType.Identity,
                        accum_out=acc[:, acc_col : acc_col + 1],
                    )

            tiles = []
            for i, (t, lo, hi, masked, col) in enumerate(work):
                g = emit_gather(t, lo, hi)
                tiles.append((t, g, lo, hi, masked, col))
                if i == 0:
                    it = nc.gpsimd.iota(
                        iota_f[:], pattern=[[1, iw]], base=iota_base,
                        channel_multiplier=0,
                        allow_small_or_imprecise_dtypes=True,
                    )
                    add_dep_helper(it.ins, idx_dma.ins, reason="iota after idx")
            n = len(tiles)
            for i, item in enumerate(tiles):
                emit_compute(*item, dve_only=(i >= n - dve_only_last))

            nc.sync.dma_start(out=res[:], in_=acc[:])

    legalize_waits(nc)
    return nc, work


def prepare_inputs(fluctuate, ivar, output, overlap_index):
    """Shard + sort samples, build per-core input maps and metadata."""
    flu = np.ascontiguousarray(fluctuate.reshape(B, L), dtype=np.float32)
    ivr = np.ascontiguousarray(ivar.reshape(B, L), dtype=np.float32)
    oup = np.ascontiguousarray(output.reshape(B, L), dtype=np.float32)
    oi = np.asarray(overlap_index)
    s_in = oi[:, 0].astype(np.int64)
    e_in = oi[:, 1].astype(np.int64)
    s_out = oi[:, 2].astype(np.int64)
    all_lens = e_in - s_in

    orders = []
    core_lens = []       # per-core lens in sorted order, [TILES, P]
    for c in range(N_CORES):
        lo = c * BPC
        lens_local = all_lens[lo : lo + BPC]
        # descending: widest tile first, so the exposed drain after the last
        # transfer runs on the narrowest tile
        order = np.argsort(-lens_local, kind="stable")
        orders.append(order)
        core_lens.append(lens_local[order].reshape(TILES, P))

    # shared tile widths (max len, rounded up to 128) and maskless base
    # widths (min len, rounded down to 128) across cores
    widths = []
    bases = []
    for t in range(TILES):
        mx = max(int(core_lens[c][t].max()) for c in range(N_CORES))
        mn = min(int(core_lens[c][t].min()) for c in range(N_CORES))
        w = min(MAX_W, -(-mx // 128) * 128)
        b = max(0, min(mn // 128 * 128, w))
        widths.append(w)
        bases.append(b)

    SEC = (BPC + 1) * L      # element offset between flu/ivr/oup sections
    in_maps = []
    for c in range(N_CORES):
        lo = c * BPC
        order = orders[c]
        rows = order.astype(np.int64)
        g = lo + order
        off_in = rows * L + s_in[g]
        off_out = rows * L + s_out[g]
        idx = np.empty((P, 3 * TILES), dtype=np.int32)
        lens_f = np.empty((P, TILES), dtype=np.float32)
        for t in range(TILES):
            sl = slice(t * P, (t + 1) * P)
            idx[:, 3 * t] = off_in[sl]
            idx[:, 3 * t + 1] = off_in[sl] + SEC
            idx[:, 3 * t + 2] = off_out[sl] + 2 * SEC
            lens_f[:, t] = all_lens[g][sl]

        end = lo + BPC
        pad = np.zeros(L, dtype=np.float32)
        parts = []
        for arr in (flu, ivr, oup):
            if end < B:
                parts.append(arr.reshape(-1)[lo * L : end * L + L])
            else:
                parts.append(
                    np.concatenate([arr.reshape(-1)[lo * L : end * L], pad])
                )
        dat = np.concatenate(parts).reshape(ROWS, L)

        in_maps.append({"dat": dat, "idx": idx, "lens": lens_f})

    return in_maps, widths, bases, core_lens


def finish(results, work, core_lens):
    """Combine per-core per-piece partial sums into the scalar mean."""
    total = 0.0
    for c in range(N_CORES):
        res = results[c]["res"].astype(np.float64)     # [P, ncol]
        sums = np.zeros((TILES, P), dtype=np.float64)
        for (t, lo, hi, masked, col) in work:
            sums[t] += res[:, col]
        lens = core_lens[c].astype(np.float64)
        total += float((sums / lens).sum())
    return np.float32(total / B)


def kernel(fluctuate, ivar, output, overlap_index, _trace=False, **_kw):
    in_maps, widths, bases, core_lens = prepare_inputs(
        fluctuate, ivar, output, overlap_index
    )
    nc, work = build_bass(widths, bases)
    out = run_bass_kernel_spmd(
        nc, in_maps, core_ids=list(range(N_CORES)), trace=_trace
    )
    result = finish(out.results, work, core_lens)
    if _trace:
        return result, out
    return result



# revision 2
# speedup vs baseline: 2.0405x; 2.0405x over previous
"""Chi2 loss over ragged windows — Trainium2 Bass kernel.

Math (per sample b of B=4096, rows of length L=4096):
    len  = e_in - s_in            (in [1024, 3072])
    chi2 = sum_{j<len} ivar[b, s_in+j] * (flu[b, s_in+j] - out[b, s_out+j])^2
    result = mean_b(chi2 / len)

Strategy: pure data-parallel over the batch, 512 samples per core on 8
cores.  Samples are globally sorted by window length (descending) and
dealt round-robin to cores, so every core sees an identical length
profile and the single SPMD program's tile widths are tight for all
cores simultaneously.

Precision staging (tolerance is 2e-2 relative): flu and out are staged
in fp8 (e3m4), and sqrt(ivar) is staged in fp16 — so the device's
weighted square becomes  t = (x - y) * sw;  chi2 += t^2,  computed as
one DVE subtract (fp8 in, fp16 out), one DVE multiply (all-fp16, 2x
mode), and one fused ACT Square-with-accumulate pass.  The sw rows are
zero outside each sample's valid window (and padded past the row end),
so the ragged tail masks itself — no iota/mask instructions.

Each 128-sample tile is fetched with one indirect DMA per array (one
offset per partition, W contiguous elements per descriptor).  Tiles go
widest-first so the 994ns/gather SWDGE descriptor-generation stays
hidden behind long transfers.  Compute is chunked so ACT chases DVE
closely; the final chunks reduce on the DVE to shorten the drain.
"""

import numpy as np
import ml_dtypes

import bass_rust
import concourse.bass as bass
import concourse.tile as tile
from concourse import mybir
from concourse.bass_utils import run_bass_kernel_spmd

B, L = 4096, 4096
N_CORES = 8
BPC = B // N_CORES          # samples per core
P = 128                     # SBUF partitions
TILES = BPC // P            # 128-sample tiles per core
MAX_W = 3072                # max window length
SW_STRIDE = L + MAX_W       # sw rows padded so gathers never cross rows

f32 = mybir.dt.float32
f16 = mybir.dt.float16
f8 = mybir.dt.float8e3
i32 = mybir.dt.int32

NP_F8 = ml_dtypes.float8_e3m4
F8_MAX = 15.0

CHUNK = 1024                # compute chunk width (columns)
DVE_ONLY_LAST = 1           # final chunks reduce on DVE instead of ACT
IO_BUFS = 3
SCR_BUFS = 4


def legalize_waits(nc):
    """This compiler build only accepts one sync wait per instruction; hoist
    extra waits into standalone single-wait EventSemaphore instructions."""
    n = 0
    for func in nc.m.functions:
        for blk in func.blocks:
            insts = blk.instructions
            out = []
            for inst in insts:
                si = inst.sync_info
                if si is not None and si.on_wait and len(si.on_wait) > 1:
                    waits = list(si.on_wait)
                    for w in waits[:-1]:
                        n += 1
                        out.append(
                            bass_rust.InstEventSemaphore(
                                name=f"splitwait_{n}_{inst.name}",
                                engine=inst.engine,
                                ins=[],
                                outs=[],
                                sync_info=mybir.SyncInfo(on_wait=[w], on_update=[]),
                            )
                        )
                    inst.sync_info = mybir.SyncInfo(
                        on_wait=[waits[-1]], on_update=list(si.on_update)
                    )
                out.append(inst)
            if len(out) != len(insts):
                blk.instructions[:] = out
    return n


def make_work(widths):
    """Compute chunks (t, lo, hi, col, dve_only), tile-major, wide tiles
    first.  The very last chunks run entirely on the DVE."""
    work = []
    col = 0
    for t in range(TILES):
        W = widths[t]
        lo = 0
        while lo < W:
            hi = min(W, lo + CHUNK)
            # avoid a tiny final chunk: fold remainders < CHUNK/2 into prev
            if W - hi < CHUNK // 2:
                hi = W
            work.append([t, lo, hi, col, False])
            col += 1
            lo = hi
    for item in work[-DVE_ONLY_LAST:]:
        item[4] = True
    return work, col


def build_bass(widths, scratch=32768):
    work, ncol = make_work(widths)

    nc = bass.Bass(dynamic_dma_scratch_size=scratch)

    xdat = nc.dram_tensor("xdat", [BPC + 1, L], f8, kind="ExternalInput")
    ydat = nc.dram_tensor("ydat", [BPC + 1, L], f8, kind="ExternalInput")
    swdat = nc.dram_tensor("swdat", [BPC, SW_STRIDE], f16, kind="ExternalInput")
    idx = nc.dram_tensor("idx", [P, 3 * TILES], i32, kind="ExternalInput")
    res = nc.dram_tensor("res", [P, ncol], f32, kind="ExternalOutput")

    with tile.TileContext(nc) as tc:
        with (
            tc.tile_pool(name="sc", bufs=1) as sc,
            tc.tile_pool(name="io", bufs=IO_BUFS) as io,
            tc.tile_pool(name="scr", bufs=SCR_BUFS) as scr,
        ):
            idx_sb = sc.tile([P, 3 * TILES], i32)
            acc = sc.tile([P, ncol], f32)

            nc.sync.dma_start(out=idx_sb[:], in_=idx[:])

            gathers = {}
            for t in range(TILES):
                W = widths[t]
                for a, (tag, dram, dt_) in enumerate(
                    (("x", xdat, f8), ("y", ydat, f8), ("sw", swdat, f16))
                ):
                    ti = io.tile([P, W], dt_, tag=tag)
                    nc.gpsimd.indirect_dma_start(
                        out=ti[:], out_offset=None, in_=dram[:],
                        in_offset=bass.IndirectOffsetOnAxis(
                            ap=idx_sb[:, 3 * t + a : 3 * t + a + 1], axis=1
                        ),
                    )
                    gathers[(t, tag)] = ti

            for (t, lo, hi, col, dve_only) in work:
                w = hi - lo
                x = gathers[(t, "x")][:, lo:hi]
                y = gathers[(t, "y")][:, lo:hi]
                sw = gathers[(t, "sw")][:, lo:hi]
                d = scr.tile([P, w], f16, tag="d")
                nc.vector.tensor_tensor(
                    out=d[:], in0=x, in1=y, op=mybir.AluOpType.subtract
                )
                nc.vector.tensor_tensor(
                    out=d[:], in0=d[:], in1=sw, op=mybir.AluOpType.mult
                )
                if dve_only:
                    sq = scr.tile([P, w], f16, tag="sq")
                    nc.vector.tensor_tensor(
                        out=sq[:], in0=d[:], in1=d[:], op=mybir.AluOpType.mult
                    )
                    nc.vector.tensor_reduce(
                        out=acc[:, col : col + 1], in_=sq[:],
                        axis=mybir.AxisListType.X, op=mybir.AluOpType.add,
                    )
                else:
                    sq = scr.tile([P, w], f16, tag="sq")
                    nc.scalar.activation(
                        out=sq[:], in_=d[:],
                        func=mybir.ActivationFunctionType.Square,
                        accum_out=acc[:, col : col + 1],
                    )

            nc.sync.dma_start(out=res[:], in_=acc[:])

    legalize_waits(nc)
    return nc, work


def prepare_inputs(fluctuate, ivar, output, overlap_index):
    """Globally sort samples by window length, deal round-robin to cores,
    stage fp8 x/y and zero-padded fp16 sqrt(ivar) per core."""
    flu = np.ascontiguousarray(fluctuate.reshape(B, L), dtype=np.float32)
    ivr = np.ascontiguousarray(ivar.reshape(B, L), dtype=np.float32)
    oup = np.ascontiguousarray(output.reshape(B, L), dtype=np.float32)
    oi = np.asarray(overlap_index)
    s_in = oi[:, 0].astype(np.int64)
    e_in = oi[:, 1].astype(np.int64)
    s_out = oi[:, 2].astype(np.int64)
    all_lens = e_in - s_in

    order = np.argsort(-all_lens, kind="stable")   # global, descending
    lens_sorted = all_lens[order]

    # tile t's width: the longest window among ranks [1024t, 1024(t+1))
    widths = []
    for t in range(TILES):
        w = int(lens_sorted[t * P * N_CORES])
        widths.append(min(MAX_W, -(-w // 32) * 32))

    x8 = NP_F8(np.clip(flu, -F8_MAX, F8_MAX))
    y8 = NP_F8(np.clip(oup, -F8_MAX, F8_MAX))
    sw16 = np.sqrt(ivr, dtype=np.float32).astype(np.float16)

    in_maps = []
    core_lens = []
    for c in range(N_CORES):
        g = order[c::N_CORES]                      # this core's samples, sorted
        lens_c = all_lens[g]
        core_lens.append(lens_c.reshape(TILES, P))

        xdat = np.zeros((BPC + 1, L), dtype=NP_F8)
        ydat = np.zeros((BPC + 1, L), dtype=NP_F8)
        xdat[:BPC] = x8[g]
        ydat[:BPC] = y8[g]
        swdat = np.zeros((BPC, SW_STRIDE), dtype=np.float16)
        rows = np.arange(BPC)
        jj = np.arange(L)
        win = (jj[None, :] >= s_in[g, None]) & (jj[None, :] < e_in[g, None])
        swc = np.where(win, sw16[g], np.float16(0))
        swdat[:, :L] = swc

        idxm = np.empty((P, 3 * TILES), dtype=np.int32)
        for t in range(TILES):
            sl = slice(t * P, (t + 1) * P)
            idxm[:, 3 * t + 0] = rows[sl] * L + s_in[g][sl]
            idxm[:, 3 * t + 1] = rows[sl] * L + s_out[g][sl]
            idxm[:, 3 * t + 2] = rows[sl] * SW_STRIDE + s_in[g][sl]

        in_maps.append({"xdat": xdat, "ydat": ydat, "swdat": swdat, "idx": idxm})

    return in_maps, widths, core_lens


def finish(results, work, core_lens):
    """Combine per-core per-chunk partial sums into the scalar mean."""
    total = 0.0
    for c in range(N_CORES):
        res = results[c]["res"].astype(np.float64)     # [P, ncol]
        sums = np.zeros((TILES, P), dtype=np.float64)
        for (t, lo, hi, col, dve_only) in work:
            sums[t] += res[:, col]
        lens = core_lens[c].astype(np.float64)
        total += float((sums / lens).sum())
    return np.float32(total / B)


def kernel(fluctuate, ivar, output, overlap_index, _trace=False, **_kw):
    in_maps, widths, core_lens = prepare_inputs(
        fluctuate, ivar, output, overlap_index
    )
    nc, work = build_bass(widths)
    out = run_bass_kernel_spmd(
        nc, in_maps, core_ids=list(range(N_CORES)), trace=_trace
    )
    result = finish(out.results, work, core_lens)
    if _trace:
        return result, out
    return result


# revision 3
# speedup vs baseline: 2.1170x; 1.0375x over previous
"""Chi2 loss over ragged windows — Trainium2 Bass kernel.

Math (per sample b of B=4096, rows of length L=4096):
    len  = e_in - s_in            (in [1024, 3072])
    chi2 = sum_{j<len} ivar[b, s_in+j] * (flu[b, s_in+j] - out[b, s_out+j])^2
    result = mean_b(chi2 / len)

Strategy: pure data-parallel over the batch, 512 samples per core on 8
cores.  Samples are globally sorted by window length (descending) and
dealt round-robin to cores, so every core sees an identical length
profile and the single SPMD program's tile widths are tight for all
cores simultaneously.

Precision staging (tolerance is 2e-2 relative): all three arrays are
staged in fp8 (e3m4, ~1.5% rms quantization).  flu and ivar share the
same window offsets, so they are interleaved element-wise into one
array and fetched with a single indirect-DMA descriptor per sample; the
ivar lanes are zeroed outside each sample's valid window (and the rows
are zero-padded past L), so the ragged tail masks itself — no
iota/mask instructions.

Per 512-column chunk the compute is spread over three engines:
  PE :  d = I @ x + (-I) @ y      (two fp8 matmuls accumulating in PSUM)
  ACT:  d2 = Square(d)            (PSUM -> SBUF fp16)
  DVE:  acc = reduce(d2 * w)      (one fused TensorTensorReduce pass)
The host divides each sample's sum by its length and means (f64).

Tiles are fetched widest-first so the 994ns/gather SWDGE descriptor
generation stays hidden behind long transfers; the first tile's gathers
are split so compute starts early.
"""

import numpy as np
import ml_dtypes

import bass_rust
import concourse.bass as bass
import concourse.tile as tile
from concourse import mybir
from concourse.bass_utils import run_bass_kernel_spmd

B, L = 4096, 4096
N_CORES = 8
BPC = B // N_CORES          # samples per core
P = 128                     # SBUF partitions
TILES = BPC // P            # 128-sample tiles per core
MAX_W = 3072                # max window length
ILV_STRIDE = 2 * (L + MAX_W)  # interleaved x/w rows, zero-padded past 2L

f32 = mybir.dt.float32
f16 = mybir.dt.float16
f8 = mybir.dt.float8e3
i32 = mybir.dt.int32

NP_F8 = ml_dtypes.float8_e3m4
F8_MAX = 15.0

CHUNK = 512                 # compute chunk width (<= one PSUM bank of f32)
FIRST_SPLIT = 512           # head split of the first tile's gathers
PSUM_BUFS = 6
IO_BUFS = 3
SCR_BUFS = 6


def legalize_waits(nc):
    """This compiler build only accepts one sync wait per instruction; hoist
    extra waits into standalone single-wait EventSemaphore instructions."""
    n = 0
    for func in nc.m.functions:
        for blk in func.blocks:
            insts = blk.instructions
            out = []
            for inst in insts:
                si = inst.sync_info
                if si is not None and si.on_wait and len(si.on_wait) > 1:
                    waits = list(si.on_wait)
                    for w in waits[:-1]:
                        n += 1
                        out.append(
                            bass_rust.InstEventSemaphore(
                                name=f"splitwait_{n}_{inst.name}",
                                engine=inst.engine,
                                ins=[],
                                outs=[],
                                sync_info=mybir.SyncInfo(on_wait=[w], on_update=[]),
                            )
                        )
                    inst.sync_info = mybir.SyncInfo(
                        on_wait=[waits[-1]], on_update=list(si.on_update)
                    )
                out.append(inst)
            if len(out) != len(insts):
                blk.instructions[:] = out
    return n


def make_chunks(W):
    """Split a tile width into balanced chunks of <= CHUNK columns."""
    n = -(-W // CHUNK)
    base = W // n
    rem = W - base * n
    out = []
    lo = 0
    for i in range(n):
        hi = lo + base + (1 if i < rem else 0)
        out.append((lo, hi))
        lo = hi
    return out


def make_work(widths):
    """Compute chunks (t, lo, hi, col), tile-major, wide tiles first."""
    work = []
    col = 0
    for t in range(TILES):
        for lo, hi in make_chunks(widths[t]):
            work.append((t, lo, hi, col))
            col += 1
    return work, col


def build_bass(widths, scratch=32768):
    work, ncol = make_work(widths)

    nc = bass.Bass(dynamic_dma_scratch_size=scratch)

    ilv = nc.dram_tensor("ilv", [BPC, ILV_STRIDE], f8, kind="ExternalInput")
    ydat = nc.dram_tensor("ydat", [BPC + 1, L], f8, kind="ExternalInput")
    idx = nc.dram_tensor("idx", [P, 3 * TILES], i32, kind="ExternalInput")
    ident = nc.dram_tensor("ident", [P, 2 * P], f8, kind="ExternalInput")
    res = nc.dram_tensor("res", [P, ncol], f32, kind="ExternalOutput")

    with tile.TileContext(nc) as tc:
        with (
            tc.tile_pool(name="sc", bufs=1) as sc,
            tc.tile_pool(name="io", bufs=IO_BUFS) as io,
            tc.tile_pool(name="scr", bufs=SCR_BUFS) as scr,
            tc.psum_pool(name="ps", bufs=PSUM_BUFS) as ps,
        ):
            idx_sb = sc.tile([P, 3 * TILES], i32)
            id_sb = sc.tile([P, 2 * P], f8)
            acc = sc.tile([P, ncol], f32)

            nc.sync.dma_start(out=idx_sb[:], in_=idx[:])
            nc.sync.dma_start(out=id_sb[:], in_=ident[:])

            def gather(dram, c, width, elem_off, tag):
                ti = io.tile([P, width], f8, tag=tag)
                nc.gpsimd.indirect_dma_start(
                    out=ti[:], out_offset=None, in_=dram[:],
                    in_offset=bass.IndirectOffsetOnAxis(
                        ap=idx_sb[:, c : c + 1], axis=1
                    ),
                    element_offset=elem_off,
                )
                return ti

            # gathers: tile 0 (widest) is split so compute starts early
            g_ilv = {}
            g_y = {}
            for t in range(TILES):
                W = widths[t]
                if t == 0 and 0 < FIRST_SPLIT < W:
                    ya = gather(ydat, 1, FIRST_SPLIT, 0, "ya")
                    ia = gather(ilv, 0, 2 * FIRST_SPLIT, 0, "ia")
                    yb = gather(ydat, 1, W - FIRST_SPLIT, FIRST_SPLIT, "y")
                    ib = gather(ilv, 0, 2 * (W - FIRST_SPLIT), 2 * FIRST_SPLIT, "i")
                    g_y[t] = (FIRST_SPLIT, ya, yb)
                    g_ilv[t] = (FIRST_SPLIT, ia, ib)
                else:
                    g_y[t] = (W, gather(ydat, 3 * t + 1, W, 0, "y"), None)
                    g_ilv[t] = (W, gather(ilv, 3 * t, 2 * W, 0, "i"), None)

            def slices(t, lo, hi):
                """(x_ap, w_ap, y_ap) for tile t columns [lo, hi)."""
                cut, ia, ib = g_ilv[t]
                cut_y, ya, yb = g_y[t]
                assert cut == cut_y
                if hi <= cut:
                    i_t, i_lo, i_hi = ia, lo, hi
                    y_ap = ya[:, lo:hi]
                else:
                    assert lo >= cut, "chunk straddles the split"
                    i_t, i_lo, i_hi = ib, lo - cut, hi - cut
                    y_ap = yb[:, lo - cut : hi - cut]
                x_ap = i_t[:, 2 * i_lo : 2 * i_hi : 2]
                w_ap = i_t[:, 2 * i_lo + 1 : 2 * i_hi : 2]
                return x_ap, w_ap, y_ap

            n = len(work)
            for k, (t, lo, hi, col) in enumerate(work):
                w = hi - lo
                x_ap, w_ap, y_ap = slices(t, lo, hi)
                d = ps.tile([P, w], f32, tag="d")
                nc.tensor.matmul(d[:], id_sb[:, 0:P], x_ap, start=True, stop=False)
                nc.tensor.matmul(d[:], id_sb[:, P : 2 * P], y_ap, start=False, stop=True)
                d2 = scr.tile([P, w], f16, tag="d2")
                if k == n - 1:
                    # drain chunk: keep the tail on one engine (DVE)
                    nc.vector.tensor_tensor(
                        out=d2[:], in0=d[:], in1=d[:], op=mybir.AluOpType.mult
                    )
                else:
                    nc.scalar.activation(
                        out=d2[:], in_=d[:],
                        func=mybir.ActivationFunctionType.Square,
                    )
                tt = scr.tile([P, w], f16, tag="tt")
                nc.vector.tensor_tensor_reduce(
                    out=tt[:], in0=d2[:], in1=w_ap, scale=1.0, scalar=0.0,
                    op0=mybir.AluOpType.mult, op1=mybir.AluOpType.add,
                    accum_out=acc[:, col : col + 1],
                )

            nc.sync.dma_start(out=res[:], in_=acc[:])

    legalize_waits(nc)
    return nc, work


def prepare_inputs(fluctuate, ivar, output, overlap_index):
    """Globally sort samples by window length, deal round-robin to cores,
    stage fp8 interleaved x/w (window-masked) and fp8 y per core."""
    flu = np.ascontiguousarray(fluctuate.reshape(B, L), dtype=np.float32)
    ivr = np.ascontiguousarray(ivar.reshape(B, L), dtype=np.float32)
    oup = np.ascontiguousarray(output.reshape(B, L), dtype=np.float32)
    oi = np.asarray(overlap_index)
    s_in = oi[:, 0].astype(np.int64)
    e_in = oi[:, 1].astype(np.int64)
    s_out = oi[:, 2].astype(np.int64)
    all_lens = e_in - s_in

    order = np.argsort(-all_lens, kind="stable")   # global, descending
    lens_sorted = all_lens[order]

    # tile t's width: the longest window among ranks [1024t, 1024(t+1))
    widths = []
    for t in range(TILES):
        w = int(lens_sorted[t * P * N_CORES])
        widths.append(min(MAX_W, -(-w // 32) * 32))

    x8 = NP_F8(np.clip(flu, -F8_MAX, F8_MAX))
    y8 = NP_F8(np.clip(oup, -F8_MAX, F8_MAX))
    w8 = NP_F8(ivr)

    ident = np.zeros((P, 2 * P), dtype=NP_F8)
    ident[:, :P] = NP_F8(np.eye(P, dtype=np.float32))
    ident[:, P:] = NP_F8(-np.eye(P, dtype=np.float32))

    jj = np.arange(L)
    in_maps = []
    core_lens = []
    for c in range(N_CORES):
        g = order[c::N_CORES]                      # this core's samples, sorted
        core_lens.append(all_lens[g].reshape(TILES, P))

        win = (jj[None, :] >= s_in[g, None]) & (jj[None, :] < e_in[g, None])
        ilv = np.zeros((BPC, ILV_STRIDE), dtype=NP_F8)
        ilv[:, 0 : 2 * L : 2] = x8[g]
        ilv[:, 1 : 2 * L : 2] = np.where(win, w8[g], NP_F8(0))
        ydat = np.zeros((BPC + 1, L), dtype=NP_F8)
        ydat[:BPC] = y8[g]

        rows = np.arange(BPC)
        idxm = np.empty((P, 3 * TILES), dtype=np.int32)
        for t in range(TILES):
            sl = slice(t * P, (t + 1) * P)
            idxm[:, 3 * t + 0] = rows[sl] * ILV_STRIDE + 2 * s_in[g][sl]
            idxm[:, 3 * t + 1] = rows[sl] * L + s_out[g][sl]
            idxm[:, 3 * t + 2] = 0

        in_maps.append(
            {"ilv": ilv, "ydat": ydat, "idx": idxm, "ident": ident}
        )

    return in_maps, widths, core_lens


def finish(results, work, core_lens):
    """Combine per-core per-chunk partial sums into the scalar mean."""
    total = 0.0
    for c in range(N_CORES):
        res = results[c]["res"].astype(np.float64)     # [P, ncol]
        sums = np.zeros((TILES, P), dtype=np.float64)
        for (t, lo, hi, col) in work:
            sums[t] += res[:, col]
        lens = core_lens[c].astype(np.float64)
        total += float((sums / lens).sum())
    return np.float32(total / B)


def kernel(fluctuate, ivar, output, overlap_index, _trace=False, **_kw):
    in_maps, widths, core_lens = prepare_inputs(
        fluctuate, ivar, output, overlap_index
    )
    nc, work = build_bass(widths)
    out = run_bass_kernel_spmd(
        nc, in_maps, core_ids=list(range(N_CORES)), trace=_trace
    )
    result = finish(out.results, work, core_lens)
    if _trace:
        return result, out
    return result
